# revision 21
# baseline (speedup 1.0000x reference)
"""GCMC GNN message-passing kernel for 8 Trainium2 NeuronCores.

Strategy (self-contained; shapes hardcoded for this problem):
  - Nodes (rows of every SpMM output) are sharded across the 8 cores; the
    small W / linear / Q weights are replicated; final bilinear scores are
    row-sharded by embed_q / embed_i and reassembled on the host.
  - Each SpMM  out = A @ X  is executed as dense PE matmuls: the host bins
    the COO edges into dense 128x128 bf16 blocks M[cl, s] = sum(val) over
    edges (col = chunk*128+cl, dst = window*128+s).  On device the full bf16
    feature tensors are resident in SBUF; for each contraction chunk the
    feature chunk halves are the stationary operand and the M slab is the
    moving operand, accumulating hidden_pre^T [d, s] into PSUM.  All DMA is
    large sequential transfers (no indirect gathers).
  - W is folded in *after* the SpMM: spmm(A, feat @ W) == spmm(A, feat) @ W.
  - BatchNorm statistics are AllReduced across cores; embed_i / embed_t are
    AllGathered for the score matmuls.
"""

import numpy as np

import concourse.bacc as bacc
import concourse.mybir as mybir
import concourse.tile as tile
from concourse.bass_utils import run_bass_kernel_spmd

N_CORES = 8
NQ, NI, NT, D, OUT = 8192, 8192, 4096, 256, 128
SQ, SI, ST = NQ // N_CORES, NI // N_CORES, NT // N_CORES  # 1024, 1024, 512
EPS = 1e-5

F32 = mybir.dt.float32
BF16 = mybir.dt.bfloat16
NP_BF16 = mybir.dt.np(BF16)

AF = mybir.ActivationFunctionType


def build_program(K=1, debug_taps=False, single_core=False, phases="all"):
    ndev = 1 if single_core else N_CORES
    nc = bacc.Bacc("TRN2", target_bir_lowering=False, debug=False, num_devices=ndev)

    # --- I/O ---
    fqT = nc.dram_tensor("fqT", [D, SQ], F32, kind="ExternalInput")
    fiT = nc.dram_tensor("fiT", [D, SI], F32, kind="ExternalInput")
    ftT = nc.dram_tensor("ftT", [D, ST], F32, kind="ExternalInput")
    xq = nc.dram_tensor("xq", [NQ, D], BF16, kind="ExternalInput")
    xi = nc.dram_tensor("xi", [NI, D], BF16, kind="ExternalInput")
    xt = nc.dram_tensor("xt", [NT, D], BF16, kind="ExternalInput")
    m1 = nc.dram_tensor("m1", [64, 128, 12, 128], BF16, kind="ExternalInput")
    m2 = nc.dram_tensor("m2", [96, 128, 8, 128], BF16, kind="ExternalInput")
    wp = nc.dram_tensor("wp", [128, 2, D], F32, kind="ExternalInput")
    wlq = nc.dram_tensor("wlq", [128, 4, OUT], F32, kind="ExternalInput")
    wli = nc.dram_tensor("wli", [128, 4, OUT], F32, kind="ExternalInput")
    wlt = nc.dram_tensor("wlt", [128, 4, OUT], F32, kind="ExternalInput")
    qm = nc.dram_tensor("qm", [OUT, OUT], F32, kind="ExternalInput")
    bnp = nc.dram_tensor("bnp", [128, 6], F32, kind="ExternalInput")
    sq_o = nc.dram_tensor("sq", [SQ, NI], F32, kind="ExternalOutput")
    st_o = nc.dram_tensor("st", [SI, NT], F32, kind="ExternalOutput")
    dbg = {}
    if debug_taps:
        for nm, shp in (
            ("dbg_hq", [128, SQ]), ("dbg_hi", [128, SI]), ("dbg_ht", [128, ST]),
            ("dbg_stats", [128, 12]), ("dbg_statsg", [128, 12]),
            ("dbg_eq", [128, SQ]), ("dbg_ei", [128, SI]), ("dbg_et", [128, ST]),
            ("dbg_hidq", [128, SQ * 2]),
        ):
            dbg[nm] = nc.dram_tensor(nm, shp, F32, kind="ExternalOutput")

    rg = [list(range(N_CORES))]

    with tile.TileContext(nc) as tc:
        with (
            tc.tile_pool(name="const", bufs=1) as cpool,
            tc.tile_pool(name="x", bufs=3) as xpool,
            tc.tile_pool(name="m", bufs=8) as mpool,
            tc.tile_pool(name="work", bufs=2) as wpool,
            tc.tile_pool(name="big", bufs=1) as bpool,
            tc.tile_pool(name="ps", space="PSUM", bufs=1) as pspool,
            tc.tile_pool(name="dram", bufs=1, space="DRAM") as dpool,
        ):
            # --- constants (loaded once) ---
            wp_t = cpool.tile([128, 2, D], F32)
            nc.sync.dma_start(out=wp_t[:], in_=wp[:])
            wl_t = {}
            for ename, wl in (("q", wlq), ("i", wli), ("t", wlt)):
                wlx = cpool.tile([128, 4, OUT], F32, name=f"wl_{ename}")
                nc.sync.dma_start(out=wlx[:], in_=wl[:])
                wl_t[ename] = wlx
            qm_t = cpool.tile([OUT, OUT], F32)
            nc.sync.dma_start(out=qm_t[:], in_=qm[:])
            bn_t = cpool.tile([128, 6], F32)
            nc.sync.dma_start(out=bn_t[:], in_=bnp[:])
            epsb = cpool.tile([128, 1], F32)
            nc.vector.memset(epsb[:], EPS)
            # resident bf16 features: [128, n_chunks, D]
            xsb = {}
            for nm, xdram, nchx in (("q", xq, 64), ("i", xi, 64), ("t", xt, 32)):
                xs = cpool.tile([128, nchx, D], BF16, name=f"xsb_{nm}")
                nc.sync.dma_start(
                    out=xs[:], in_=xdram[:].rearrange("(a p) d -> p a d", p=128)
                )
                xsb[nm] = xs
            fT_t = {}
            for ename, fT, rows in (("q", fqT, SQ), ("i", fiT, SI), ("t", ftT, ST)):
                f0 = cpool.tile([128, rows], F32, name=f"fT0_{ename}")
                f1 = cpool.tile([128, rows], F32, name=f"fT1_{ename}")
                nc.sync.dma_start(out=f0[:], in_=fT[0:128, :])
                nc.sync.dma_start(out=f1[:], in_=fT[128:256, :])
                fT_t[ename] = (f0, f1)

            for rep in range(K):
                # hidden^T result tiles (2 dim-halves each)
                hqT = [bpool.tile([128, SQ], F32, tag=f"hqT{h}", name=f"hqT{h}")
                       for h in range(2)]
                htT = [bpool.tile([128, ST], F32, tag=f"htT{h}", name=f"htT{h}")
                       for h in range(2)]
                hiT = [bpool.tile([128, SI], F32, tag=f"hiT{h}", name=f"hiT{h}")
                       for h in range(2)]

                def post_group(pgrp, nwin, dstT, wbase):
                    """pgrp: per-dim-half PSUM tiles [128, nwin*128] holding
                    hidden_pre^T; -> relu(hpre @ W)^T into dstT halves."""
                    hpT = wpool.tile([128, 2, 512], F32, tag="hpT", name="hpT")
                    for dh in range(2):
                        nc.vector.tensor_copy(
                            out=hpT[:, dh, 0:nwin * 128], in_=pgrp[dh][:]
                        )
                    for w4 in range(nwin):
                        pwm = pspool.tile(
                            [128, 2, 128], F32, tag="post", bufs=2, name="pwm"
                        )
                        for d2h in range(2):
                            for d1h in range(2):
                                nc.tensor.matmul(
                                    out=pwm[:, d2h, :],
                                    lhsT=wp_t[:, d1h, d2h * 128:(d2h + 1) * 128],
                                    rhs=hpT[:, d1h, w4 * 128:(w4 + 1) * 128],
                                    start=(d1h == 0),
                                    stop=(d1h == 1),
                                )
                            nc.scalar.activation(
                                out=dstT[d2h][:, (wbase + w4) * 128:(wbase + w4 + 1) * 128],
                                in_=pwm[:, d2h, :],
                                func=AF.Relu,
                            )

                # ---- sub-pass 1: hidden_q windows 0..7 over feature_i chunks
                pA = [pspool.tile([128, 512], F32, tag=f"p1_{dh}", name=f"pA{dh}")
                      for dh in range(2)]
                pB = [pspool.tile([128, 512], F32, tag=f"p2_{dh}", name=f"pB{dh}")
                      for dh in range(2)]
                for cc in range(64):
                    mc = mpool.tile([128, 8, 128], BF16, tag="mc", name="mc")
                    nc.sync.dma_start(out=mc[:], in_=m1[cc][:, 0:8, :])
                    for dh in range(2):
                        lhs = xsb["i"][:, cc, dh * 128:(dh + 1) * 128]
                        nc.tensor.matmul(
                            out=pA[dh][:], lhsT=lhs, rhs=mc[:, 0:4, :],
                            start=(cc == 0), stop=(cc == 63),
                        )
                        nc.tensor.matmul(
                            out=pB[dh][:], lhsT=lhs, rhs=mc[:, 4:8, :],
                            start=(cc == 0), stop=(cc == 63),
                        )
                post_group(pA, 4, hqT, 0)
                post_group(pB, 4, hqT, 4)

                # ---- sub-pass 2: hidden_t w0..3 (feature_i) + hidden_i w0..7
                #      (feature_q then feature_t chunks)
                pT = [pspool.tile([128, 512], F32, tag=f"p1_{dh}", name=f"pT{dh}")
                      for dh in range(2)]
                pC = [pspool.tile([128, 512], F32, tag=f"p2_{dh}", name=f"pC{dh}")
                      for dh in range(2)]
                pD = [pspool.tile([128, 512], F32, tag=f"p3_{dh}", name=f"pD{dh}")
                      for dh in range(2)]
                for cc in range(64):
                    mc = mpool.tile([128, 8, 128], BF16, tag="mc", name="mct")
                    nc.sync.dma_start(out=mc[:, 0:4, :], in_=m1[cc][:, 8:12, :])
                    for dh in range(2):
                        lhs = xsb["i"][:, cc, dh * 128:(dh + 1) * 128]
                        nc.tensor.matmul(
                            out=pT[dh][:], lhsT=lhs, rhs=mc[:, 0:4, :],
                            start=(cc == 0), stop=(cc == 63),
                        )
                for cc in range(96):
                    mc = mpool.tile([128, 8, 128], BF16, tag="mc", name="mci")
                    nc.sync.dma_start(out=mc[:], in_=m2[cc])
                    xn, ccl = ("q", cc) if cc < 64 else ("t", cc - 64)
                    for dh in range(2):
                        lhs = xsb[xn][:, ccl, dh * 128:(dh + 1) * 128]
                        nc.tensor.matmul(
                            out=pC[dh][:], lhsT=lhs, rhs=mc[:, 0:4, :],
                            start=(cc == 0), stop=(cc == 95),
                        )
                        nc.tensor.matmul(
                            out=pD[dh][:], lhsT=lhs, rhs=mc[:, 4:8, :],
                            start=(cc == 0), stop=(cc == 95),
                        )
                post_group(pT, 4, htT, 0)
                post_group(pC, 4, hiT, 0)
                post_group(pD, 4, hiT, 4)

                if debug_taps and rep == 0:
                    nc.sync.dma_start(out=dbg["dbg_hidq"][:, 0:SQ], in_=hqT[0][:])
                    nc.sync.dma_start(out=dbg["dbg_hidq"][:, SQ:2 * SQ], in_=hqT[1][:])
                if phases == "spmm":
                    ob = xpool.tile([128, 512], F32, tag="ob", bufs=4, name="obx")
                    nc.vector.tensor_copy(out=ob[:], in_=hqT[0][:, 0:512])
                    nc.sync.dma_start(out=sq_o[0:128, 0:512], in_=ob[:])
                    continue

                # =========== Linear (+ BN stats) per entity ===========
                stats = wpool.tile([128, 12], F32, tag="stats", name="stats")
                nc.vector.memset(stats[:], 0.0)
                scratch = wpool.tile([128, 512], F32, tag="scratch", name="scratch")
                hT_pre = {}
                ent_info = (("q", SQ, hqT, 0), ("i", SI, hiT, 4), ("t", ST, htT, 8))
                for ename, rows, hT, sbase in ent_info:
                    hp = bpool.tile(
                        [128, rows], F32,
                        tag={"q": "hqT0", "i": "hiT0", "t": "htT0"}[ename],
                        name=f"hT_{ename}",
                    )
                    hT_pre[ename] = hp
                    chunks = [hT[0], hT[1], fT_t[ename][0], fT_t[ename][1]]
                    nrt = rows // 512
                    for rt in range(nrt):
                        ph = pspool.tile([128, 512], F32, tag="post", bufs=2, name="lin")
                        for ci4 in range(4):
                            nc.tensor.matmul(
                                out=ph[:],
                                lhsT=wl_t[ename][:, ci4, :],
                                rhs=chunks[ci4][:, rt * 512:(rt + 1) * 512],
                                start=(ci4 == 0),
                                stop=(ci4 == 3),
                            )
                        nc.scalar.activation(
                            out=hp[:, rt * 512:(rt + 1) * 512],
                            in_=ph[:],
                            func=AF.Identity,
                            accum_out=stats[:, sbase + rt:sbase + rt + 1],
                        )
                    for rt in range(nrt):
                        nc.scalar.activation(
                            out=scratch[:],
                            in_=hp[:, rt * 512:(rt + 1) * 512],
                            func=AF.Square,
                            accum_out=stats[:, sbase + 2 + rt:sbase + 3 + rt],
                        )

                if debug_taps and rep == 0:
                    nc.sync.dma_start(out=dbg["dbg_hq"][:], in_=hT_pre["q"][:])
                    nc.sync.dma_start(out=dbg["dbg_hi"][:], in_=hT_pre["i"][:])
                    nc.sync.dma_start(out=dbg["dbg_ht"][:], in_=hT_pre["t"][:])
                    nc.sync.dma_start(out=dbg["dbg_stats"][:], in_=stats[:])

                # =========== BN stats AllReduce ===========
                st_in = dpool.tile([128, 12], F32, tag="st_in", name="st_in")
                st_out = dpool.tile(
                    [128, 12], F32, tag="st_out", name="st_out",
                    addr_space="Local" if single_core else "Shared",
                )
                nc.gpsimd.dma_start(out=st_in[:], in_=stats[:])
                if single_core:
                    nc.gpsimd.dma_start(out=st_out[:], in_=st_in[:])
                else:
                    nc.gpsimd.collective_compute(
                        "AllReduce",
                        mybir.AluOpType.add,
                        replica_groups=rg,
                        ins=[st_in.opt()],
                        outs=[st_out.opt()],
                    )
                statsg = wpool.tile([128, 12], F32, tag="statsg", name="statsg")
                nc.sync.dma_start(out=statsg[:], in_=st_out[:])

                # per entity: scale/shift + fused BN+relu
                emb = {}
                for ename, rows, _hT, sbase in ent_info:
                    n_nodes = float(rows * N_CORES)
                    sh = wpool.tile([128, 4], F32, tag=f"bn_{ename}", name=f"bn_{ename}")
                    # sh cols: 0 = mu, 1 = E[h^2], 2 = scale, 3 = shift
                    if rows == 512:
                        nc.vector.tensor_scalar_mul(
                            sh[:, 0:1], statsg[:, sbase:sbase + 1], 1.0 / n_nodes
                        )
                        nc.vector.tensor_scalar_mul(
                            sh[:, 1:2], statsg[:, sbase + 2:sbase + 3], 1.0 / n_nodes
                        )
                    else:
                        nc.vector.tensor_tensor(
                            out=sh[:, 0:1],
                            in0=statsg[:, sbase:sbase + 1],
                            in1=statsg[:, sbase + 1:sbase + 2],
                            op=mybir.AluOpType.add,
                        )
                        nc.vector.tensor_scalar_mul(sh[:, 0:1], sh[:, 0:1], 1.0 / n_nodes)
                        nc.vector.tensor_tensor(
                            out=sh[:, 1:2],
                            in0=statsg[:, sbase + 2:sbase + 3],
                            in1=statsg[:, sbase + 3:sbase + 4],
                            op=mybir.AluOpType.add,
                        )
                        nc.vector.tensor_scalar_mul(sh[:, 1:2], sh[:, 1:2], 1.0 / n_nodes)
                    musq = wpool.tile([128, 1], F32, tag="musq", name="musq")
                    nc.vector.tensor_tensor(
                        out=musq[:], in0=sh[:, 0:1], in1=sh[:, 0:1],
                        op=mybir.AluOpType.mult,
                    )
                    var = wpool.tile([128, 1], F32, tag="var", name="var")
                    nc.vector.tensor_tensor(
                        out=var[:], in0=sh[:, 1:2], in1=musq[:],
                        op=mybir.AluOpType.subtract,
                    )
                    sd = wpool.tile([128, 1], F32, tag="sd", name="sd")
                    nc.scalar.activation(out=sd[:], in_=var[:], func=AF.Sqrt, bias=epsb[:])
                    rstd = wpool.tile([128, 1], F32, tag="rstd", name="rstd")
                    nc.vector.reciprocal(out=rstd[:], in_=sd[:])
                    e2 = {"q": 0, "i": 2, "t": 4}[ename]
                    nc.vector.tensor_tensor(
                        out=sh[:, 2:3], in0=bn_t[:, e2:e2 + 1], in1=rstd[:],
                        op=mybir.AluOpType.mult,
                    )
                    tmp2 = wpool.tile([128, 1], F32, tag="tmp2", name="tmp2")
                    nc.vector.tensor_tensor(
                        out=tmp2[:], in0=sh[:, 0:1], in1=sh[:, 2:3],
                        op=mybir.AluOpType.mult,
                    )
                    nc.vector.tensor_tensor(
                        out=sh[:, 3:4], in0=bn_t[:, e2 + 1:e2 + 2], in1=tmp2[:],
                        op=mybir.AluOpType.subtract,
                    )
                    eT = bpool.tile(
                        [128, rows], F32,
                        tag={"q": "hqT1", "i": "hiT1", "t": "htT1"}[ename],
                        name=f"eT_{ename}",
                    )
                    emb[ename] = eT
                    nc.scalar.activation(
                        out=eT[:],
                        in_=hT_pre[ename][:],
                        func=AF.Relu,
                        scale=sh[:, 2:3],
                        bias=sh[:, 3:4],
                    )

                if debug_taps and rep == 0:
                    nc.sync.dma_start(out=dbg["dbg_statsg"][:], in_=statsg[:])
                    nc.sync.dma_start(out=dbg["dbg_eq"][:], in_=emb["q"][:])
                    nc.sync.dma_start(out=dbg["dbg_ei"][:], in_=emb["i"][:])
                    nc.sync.dma_start(out=dbg["dbg_et"][:], in_=emb["t"][:])
                if phases == "embed":
                    ob = xpool.tile([128, 512], F32, tag="ob", bufs=4, name="obe")
                    nc.vector.tensor_copy(out=ob[:], in_=emb["q"][:, 0:512])
                    nc.sync.dma_start(out=sq_o[0:128, 0:512], in_=ob[:])
                    continue

                # =========== AllGather embed_i ++ embed_t (single op) ====
                ag_in = dpool.tile([128, SI + ST], F32, tag="ag_in", name="ag_in")
                nc.gpsimd.dma_start(out=ag_in[:, 0:SI], in_=emb["i"][:])
                nc.gpsimd.dma_start(out=ag_in[:, SI:SI + ST], in_=emb["t"][:])
                ag_space = "Local" if single_core else "Shared"
                ag = dpool.tile(
                    [N_CORES * 128, SI + ST], F32, tag="ag", name="ag",
                    addr_space=ag_space,
                )
                if single_core:
                    for rr in range(N_CORES):
                        nc.gpsimd.dma_start(
                            out=ag[rr * 128:(rr + 1) * 128, :], in_=ag_in[:]
                        )
                else:
                    nc.gpsimd.collective_compute(
                        "AllGather", mybir.AluOpType.bypass, replica_groups=rg,
                        ins=[ag_in.opt()], outs=[ag.opt()],
                    )

                # =========== u = Q^T @ embed^T ===========
                uT = {}
                for ename in ("q", "i"):
                    u = bpool.tile(
                        [128, 1024], F32,
                        tag={"q": "hqT0", "i": "hiT0"}[ename],
                        name=f"uT_{ename}",
                    )
                    uT[ename] = u
                    for half in range(2):
                        pu = pspool.tile(
                            [128, 512], F32, tag=f"p{1 + half}_0", name="pu"
                        )
                        nc.tensor.matmul(
                            out=pu[:],
                            lhsT=qm_t[:],
                            rhs=emb[ename][:, half * 512:(half + 1) * 512],
                            start=True,
                            stop=True,
                        )
                        nc.vector.tensor_copy(out=u[:, half * 512:(half + 1) * 512], in_=pu[:])

                # =========== scores (batched output DMA over 4 row-tiles) ====
                psum_tags = ["p1_0", "p1_1", "p2_0", "p2_1", "p3_0", "p3_1"]
                cnt = 0
                for r in range(N_CORES):
                    for j2 in range(2):
                        rb = xpool.tile([128, 512], F32, tag="rb", name="rb")
                        nc.sync.dma_start(
                            out=rb[:],
                            in_=ag[r * 128:(r + 1) * 128, j2 * 512:(j2 + 1) * 512],
                        )
                        for rtg in range(2):
                            ob4 = xpool.tile([128, 4, 512], F32, tag="ob4", bufs=3, name="ob4")
                            for rt4 in range(4):
                                rt = rtg * 4 + rt4
                                pq = pspool.tile(
                                    [128, 512], F32, tag=psum_tags[cnt % 6], name="pq"
                                )
                                nc.tensor.matmul(
                                    out=pq[:],
                                    lhsT=uT["q"][:, rt * 128:(rt + 1) * 128],
                                    rhs=rb[:],
                                    start=True,
                                    stop=True,
                                )
                                nc.vector.tensor_copy(out=ob4[:, rt4, :], in_=pq[:])
                                cnt += 1
                            nc.sync.dma_start(
                                out=sq_o[
                                    rtg * 512:(rtg + 1) * 512,
                                    r * SI + j2 * 512:r * SI + (j2 + 1) * 512,
                                ].rearrange("(a p) c -> p a c", p=128),
                                in_=ob4[:],
                            )
                for r in range(N_CORES):
                    rb = xpool.tile([128, 512], F32, tag="rb", name="rbt")
                    nc.sync.dma_start(
                        out=rb[:], in_=ag[r * 128:(r + 1) * 128, SI:SI + ST]
                    )
                    for rtg in range(2):
                        ob4 = xpool.tile([128, 4, 512], F32, tag="ob4", bufs=3, name="ob4t")
                        for rt4 in range(4):
                            rt = rtg * 4 + rt4
                            pq = pspool.tile(
                                [128, 512], F32, tag=psum_tags[cnt % 6], name="pqt"
                            )
                            nc.tensor.matmul(
                                out=pq[:],
                                lhsT=uT["i"][:, rt * 128:(rt + 1) * 128],
                                rhs=rb[:],
                                start=True,
                                stop=True,
                            )
                            nc.vector.tensor_copy(out=ob4[:, rt4, :], in_=pq[:])
                            cnt += 1
                        nc.sync.dma_start(
                            out=st_o[
                                rtg * 512:(rtg + 1) * 512,
                                r * ST:(r + 1) * ST,
                            ].rearrange("(a p) c -> p a c", p=128),
                            in_=ob4[:],
                        )

    nc.compile()
    return nc


def host_prepare(inputs):
    """Build per-core input maps from full inputs."""
    inp = {k: np.asarray(v) for k, v in inputs.items()}
    fq = inp["feature_q"].astype(np.float32)
    fi = inp["feature_i"].astype(np.float32)
    ft = inp["feature_t"].astype(np.float32)
    qi_src = inp["qi_src"].astype(np.int64)
    qi_dst = inp["qi_dst"].astype(np.int64)
    qi_val = inp["qi_val"].astype(np.float32)
    it_src = inp["it_src"].astype(np.int64)
    it_dst = inp["it_dst"].astype(np.int64)
    it_val = inp["it_val"].astype(np.float32)

    # M1: [core, chunk(feat_i 64), cl, w(12), s]
    M1 = np.zeros((N_CORES, 64, 128, 12, 128), np.float32)
    np.add.at(
        M1,
        (qi_src // SQ, qi_dst // 128, qi_dst % 128, (qi_src % SQ) // 128, qi_src % 128),
        qi_val,
    )
    np.add.at(
        M1,
        (it_dst // ST, it_src // 128, it_src % 128, 8 + (it_dst % ST) // 128, it_dst % 128),
        it_val,
    )
    # M2: [core, chunk(feat_q 64 + feat_t 32), cl, w(8), s]
    M2 = np.zeros((N_CORES, 96, 128, 8, 128), np.float32)
    np.add.at(
        M2,
        (qi_dst // SI, qi_src // 128, qi_src % 128, (qi_dst % SI) // 128, qi_dst % 128),
        qi_val,
    )
    np.add.at(
        M2,
        (it_src // SI, 64 + it_dst // 128, it_dst % 128, (it_src % SI) // 128, it_src % 128),
        it_val,
    )
    M1 = M1.astype(NP_BF16)
    M2 = M2.astype(NP_BF16)

    W = inp["W"].astype(np.float32)
    wp = W.reshape(2, 128, D).transpose(1, 0, 2).copy()
    wls = {}
    for ename in ("q", "i", "t"):
        Wl = inp[f"W{ename}"].astype(np.float32)  # [128, 512]
        wls[ename] = Wl.T.reshape(4, 128, OUT).transpose(1, 0, 2).copy()
    bnp = np.stack(
        [
            inp["gq"], inp["betaq"], inp["gi"], inp["betai"], inp["gt"], inp["betat"],
        ],
        axis=1,
    ).astype(np.float32)  # [128, 6]
    qmat = inp["Q"].astype(np.float32)

    xq_b = fq.astype(NP_BF16)
    xi_b = fi.astype(NP_BF16)
    xt_b = ft.astype(NP_BF16)
    fqT = np.ascontiguousarray(fq.T)
    fiT = np.ascontiguousarray(fi.T)
    ftT = np.ascontiguousarray(ft.T)

    in_maps = []
    for c in range(N_CORES):
        in_maps.append(
            {
                "fqT": np.ascontiguousarray(fqT[:, c * SQ:(c + 1) * SQ]),
                "fiT": np.ascontiguousarray(fiT[:, c * SI:(c + 1) * SI]),
                "ftT": np.ascontiguousarray(ftT[:, c * ST:(c + 1) * ST]),
                "xq": xq_b,
                "xi": xi_b,
                "xt": xt_b,
                "m1": np.ascontiguousarray(M1[c]),
                "m2": np.ascontiguousarray(M2[c]),
                "wp": wp,
                "wlq": wls["q"],
                "wli": wls["i"],
                "wlt": wls["t"],
                "qm": qmat,
                "bnp": bnp,
            }
        )
    return in_maps


def assemble(results):
    score_qi = np.concatenate([results[c]["sq"] for c in range(N_CORES)], axis=0)
    score_it = np.concatenate([results[c]["st"] for c in range(N_CORES)], axis=0)
    return score_qi, score_it


_PROGRAM = None


def kernel(**inputs):
    global _PROGRAM
    if _PROGRAM is None:
        _PROGRAM = build_program(K=1)
    in_maps = host_prepare(inputs)
    res = run_bass_kernel_spmd(_PROGRAM, in_maps, list(range(N_CORES)))
    return assemble(res.results)
